# revision 12
# baseline (speedup 1.0000x reference)
"""Trainium2 Bass kernel for MinimalLightningIndexer.

out[b,t,s] = relu((x@Wq)[b,t] . (x@Wk)[b,s]) * (x@Ww)[b,t]

Sharding: 8 cores = 4 batches x 2 query-halves, keys replicated. Each
core loads x[b].T (16 MB bf16) with its own t-half tokens permuted to
the front (one SPMD program serves all cores; the host un-permutes
score columns on assembly). q/w/k are projected in ONE fused pass per
512-token chunk: stationary [Wq|Ww|0|Wk] (48 wide, k 32-aligned), so a
single VectorE copy [48,512] drains each chunk's PSUM. Scores run as
row-tiled matmul pairs (two concurrent K=16 matmuls via PE row groups
0 and 1) into [128,1024] PSUM tiles as soon as each key chunk lands,
so output DMA starts ~10us in and the kernel stays DMA-bound
(16 MB in + 16 MB out per core at ~358 GB/s).

Epilogue relu*gate is split across three engines to stay under the DMA
roofline: VectorE fused tensor_scalar (max 0, mult w) for some units;
ScalarE relu + GpSimd bf16 mul for the rest.
"""

import sys

if "/opt/trn_rl_repo" not in sys.path:
    sys.path.insert(0, "/opt/trn_rl_repo")

import numpy as np

import concourse.bacc as bacc
import concourse.bass as bass
import concourse.mybir as mybir
import concourse.tile as tile
from concourse.bass_utils import run_bass_kernel_spmd

B, S, D = 4, 4096, 2048
IDX = 16
# fused projection stationary columns: q @ 0-15, k @ 32-47, w @ 64
# (32-aligned groups: engine PSUM reads must start 32-aligned)
QWK = 65
N_CORES = 8
T = S // 2            # own query tokens per core
DC = D // 128         # 16 d-chunks of 128
SC = S // 512         # 8 token chunks (0-3 own, 4-7 peer)
TT = T // 128         # 16 t-tiles

# epilogue unit (ti*4+pg) assignment: True -> ScalarE relu + GpSimd mul,
# False -> fused VectorE tensor_scalar.  36/64 on the scalar path.
_S_PATH = [u % 16 < 9 for u in range(64)]

_CACHE = {}


def _build_nc():
    if "nc" in _CACHE:
        return _CACHE["nc"]
    f32 = mybir.dt.float32
    bf16 = mybir.dt.bfloat16
    nc = bacc.Bacc("TRN2", target_bir_lowering=False, debug=False,
                   num_devices=N_CORES)
    xt = nc.dram_tensor("xt", [D, S], bf16, kind="ExternalInput").ap()
    wqwk = nc.dram_tensor("wqwk", [D, QWK], bf16, kind="ExternalInput").ap()
    o = nc.dram_tensor("o", [T, S], bf16, kind="ExternalOutput").ap()

    with tile.TileContext(nc) as tc:
        with (
            tc.tile_pool(name="const", bufs=1) as cpool,
            tc.tile_pool(name="slab", bufs=3) as slab_pool,
            tc.tile_pool(name="osb", bufs=3) as out_pool,
            tc.tile_pool(name="pqwk", bufs=2, space="PSUM") as pq_pool,
            tc.tile_pool(name="ps", bufs=3, space="PSUM") as ps_pool,
        ):
            # --- persistent tensors ---
            wqwk_sb = cpool.tile([128, DC * QWK], bf16, tag="wqwk_sb")
            nc.sync.dma_start(
                out=wqwk_sb[:],
                in_=wqwk.rearrange("(kd p) i -> p kd i", p=128),
            )
            # projection landing tile: rows 0-15 q (cols 0-2047 valid),
            # row 16 w, rows 32-47 k -> used directly as key band 1
            ab = cpool.tile([48, S], bf16, tag="ab")
            # key band 0 (partitions 0-15, replica of ab[32:48])
            kt0 = cpool.tile([16, S], bf16, tag="kt0")
            # q band 1 (partitions 32-47, replica of ab[0:16])
            qb1 = cpool.tile([48, T], bf16, tag="qb1")
            # gate row (f32: tensor_scalar scalars must be f32) and its
            # [128, TT] transpose
            wrow = cpool.tile([1, T], f32, tag="wrow")
            w_col = cpool.tile([128, TT], f32, tag="w_col")

            # --- projections per 512-token chunk ---
            for j in range(SC):
                slab = slab_pool.tile([128, DC * 512], bf16, tag="slab")
                nc.sync.dma_start(
                    out=slab[:],
                    in_=xt[:, j * 512:(j + 1) * 512].rearrange(
                        "(kd p) s -> p kd s", p=128),
                )
                slab_v = slab[:].rearrange("p (kd t) -> p kd t", kd=DC)

                psq = pq_pool.tile([QWK, 512], f32, tag="psq")
                for kd in range(DC):
                    nc.tensor.matmul(
                        psq[:],
                        wqwk_sb[:, kd * QWK:(kd + 1) * QWK],
                        slab_v[:, kd, :],
                        start=(kd == 0), stop=(kd == DC - 1),
                    )
                cols = slice(j * 512, (j + 1) * 512)
                # one VectorE copy drains q+w+k (peer chunks: q/w junk,
                # never read)
                nc.vector.tensor_copy(ab[0:48, cols], psq[0:48, :])
                if j < 4:
                    nc.vector.tensor_copy(wrow[0:1, cols], psq[64:65, :])
                # k band 0 replica via DMA (off the compute engines)
                nc.sync.dma_start(out=kt0[0:16, cols], in_=ab[32:48, cols])
                if j < 4:
                    # q band 1 replica
                    nc.sync.dma_start(
                        out=qb1[32:48, cols], in_=ab[0:16, cols],
                    )
                    # transpose gate pieces for this chunk's t-tiles
                    for t in range(4):
                        ti = j * 4 + t
                        nc.sync.dma_start(
                            out=w_col[:, ti:ti + 1],
                            in_=wrow[0:1, ti * 128:(ti + 1) * 128],
                        )

            # --- scores ---
            for ti in range(TT):
                osb = out_pool.tile([128, S], bf16, tag="osb")
                for pg in range(4):
                    pss = ps_pool.tile([128, 1024], f32, tag="pss")
                    for h in range(2):
                        sc = 2 * pg + h
                        if h == 0:
                            lhsT = ab[0:16, ti * 128:(ti + 1) * 128]
                            rhs = kt0[0:16, sc * 512:(sc + 1) * 512]
                        else:
                            lhsT = qb1[32:48, ti * 128:(ti + 1) * 128]
                            rhs = ab[32:48, sc * 512:(sc + 1) * 512]
                        nc.tensor.matmul(
                            pss[:, h * 512:(h + 1) * 512], lhsT, rhs,
                            start=True, stop=True,
                        )
                    oslice = osb[:, 2 * pg * 512:(2 * pg + 2) * 512]
                    if _S_PATH[ti * 4 + pg]:
                        nc.scalar.activation(
                            oslice, pss[:],
                            mybir.ActivationFunctionType.Relu,
                        )
                        nc.gpsimd.tensor_scalar_mul(
                            out=oslice, in0=oslice,
                            scalar1=w_col[:, ti:ti + 1],
                        )
                    else:
                        nc.vector.tensor_scalar(
                            oslice, pss[:],
                            scalar1=0.0,
                            scalar2=w_col[:, ti:ti + 1],
                            op0=mybir.AluOpType.max,
                            op1=mybir.AluOpType.mult,
                        )
                nc.sync.dma_start(
                    out=o[ti * 128:(ti + 1) * 128, :],
                    in_=osb[:],
                )
    nc.compile()
    _CACHE["nc"] = nc
    return nc


def _make_in_maps(x, Wq, Wk, Ww):
    import ml_dtypes
    bf = ml_dtypes.bfloat16
    wqwk = np.zeros((D, QWK), dtype=np.float32)
    wqwk[:, 0:16] = Wq
    wqwk[:, 32:48] = Wk
    wqwk[:, 64:65] = Ww
    wqwk = np.ascontiguousarray(wqwk).astype(bf)
    xbf = x.astype(bf)
    in_maps = []
    for c in range(N_CORES):
        b, h = c // 2, c % 2
        own = xbf[b, h * T:(h + 1) * T, :]
        oth = xbf[b, (1 - h) * T:(2 - h) * T, :]
        xtc = np.ascontiguousarray(np.concatenate([own, oth], axis=0).T)
        in_maps.append({"xt": xtc, "wqwk": wqwk})
    return in_maps


def _assemble(results):
    out = np.empty((B, S, S), dtype=np.float32)
    for c in range(N_CORES):
        b, h = c // 2, c % 2
        oc = np.asarray(results[c]["o"], dtype=np.float32)
        if h == 1:
            oc = np.concatenate([oc[:, T:], oc[:, :T]], axis=1)
        out[b, h * T:(h + 1) * T, :] = oc
    return out


def kernel(x, Wq, Wk, Ww, _trace_kwargs=None):
    nc = _build_nc()
    in_maps = _make_in_maps(np.asarray(x, dtype=np.float32),
                            np.asarray(Wq, dtype=np.float32),
                            np.asarray(Wk, dtype=np.float32),
                            np.asarray(Ww, dtype=np.float32))
    kw = _trace_kwargs or {}
    res = run_bass_kernel_spmd(nc, in_maps, list(range(N_CORES)), **kw)
    out = _assemble(res.results)
    if _trace_kwargs is not None:
        return out, res
    return out


# revision 13
# speedup vs baseline: 3.9341x; 3.9341x over previous
"""Trainium2 Bass kernel for MinimalLightningIndexer.

out[b,t,s] = relu((x@Wq)[b,t] . (x@Wk)[b,s]) * (x@Ww)[b,t]

Sharding: 8 cores = 4 batches x 2 query-halves, keys replicated. Each
core loads x[b].T (16 MB bf16) with its own t-half tokens permuted to
the front (one SPMD program serves all cores; the host un-permutes
score columns on assembly). q/w/k are projected in ONE fused pass per
512-token chunk: stationary [Wq|Ww|0|Wk] (48 wide, k 32-aligned), so a
single VectorE copy [48,512] drains each chunk's PSUM. Scores run as
row-tiled matmul pairs (two concurrent K=16 matmuls via PE row groups
0 and 1) into [128,1024] PSUM tiles as soon as each key chunk lands,
so output DMA starts ~10us in and the kernel stays DMA-bound
(16 MB in + 16 MB out per core at ~358 GB/s).

Epilogue relu*gate is split across three engines to stay under the DMA
roofline: VectorE fused tensor_scalar (max 0, mult w) for some units;
ScalarE relu + GpSimd bf16 mul for the rest.
"""

import sys

if "/opt/trn_rl_repo" not in sys.path:
    sys.path.insert(0, "/opt/trn_rl_repo")

import numpy as np

import concourse.bacc as bacc
import concourse.bass as bass
import concourse.mybir as mybir
import concourse.tile as tile
from concourse.bass_utils import run_bass_kernel_spmd

B, S, D = 4, 4096, 2048
IDX = 16
# fused projection stationary columns: q @ 0-15, k @ 32-47, w @ 64
# (32-aligned groups: engine PSUM reads must start 32-aligned)
QWK = 65
N_CORES = 8
T = S // 2            # own query tokens per core
DC = D // 128         # 16 d-chunks of 128
SC = S // 512         # 8 token chunks (0-3 own, 4-7 peer)
TT = T // 128         # 16 t-tiles

# epilogue unit (ti*4+pg) assignment: True -> ScalarE relu + VectorE mul,
# False -> fused VectorE tensor_scalar.  44/64 on the scalar path
# balances ScalarE (~1.0us/relu) against VectorE (~1.2us fused,
# ~0.33us bf16 mul).
_S_PATH = [u % 16 < 11 for u in range(64)]

_CACHE = {}


def _build_nc():
    if "nc" in _CACHE:
        return _CACHE["nc"]
    f32 = mybir.dt.float32
    bf16 = mybir.dt.bfloat16
    nc = bacc.Bacc("TRN2", target_bir_lowering=False, debug=False,
                   num_devices=N_CORES)
    xt = nc.dram_tensor("xt", [D, S], bf16, kind="ExternalInput").ap()
    wqwk = nc.dram_tensor("wqwk", [D, QWK], bf16, kind="ExternalInput").ap()
    o = nc.dram_tensor("o", [T, S], bf16, kind="ExternalOutput").ap()

    with tile.TileContext(nc) as tc:
        with (
            tc.tile_pool(name="const", bufs=1) as cpool,
            tc.tile_pool(name="slab", bufs=3) as slab_pool,
            tc.tile_pool(name="osb", bufs=3) as out_pool,
            tc.tile_pool(name="pqwk", bufs=2, space="PSUM") as pq_pool,
            tc.tile_pool(name="ps", bufs=3, space="PSUM") as ps_pool,
        ):
            # --- persistent tensors ---
            wqwk_sb = cpool.tile([128, DC * QWK], bf16, tag="wqwk_sb")
            nc.sync.dma_start(
                out=wqwk_sb[:],
                in_=wqwk.rearrange("(kd p) i -> p kd i", p=128),
            )
            # projection landing tile: rows 0-15 q (cols 0-2047 valid),
            # row 16 w, rows 32-47 k -> used directly as key band 1
            ab = cpool.tile([48, S], bf16, tag="ab")
            # key band 0 (partitions 0-15, replica of ab[32:48])
            kt0 = cpool.tile([16, S], bf16, tag="kt0")
            # q band 1 (partitions 32-47, replica of ab[0:16])
            qb1 = cpool.tile([48, T], bf16, tag="qb1")
            # gate row (f32: tensor_scalar scalars must be f32) and its
            # [128, TT] transpose
            wrow = cpool.tile([1, T], f32, tag="wrow")
            w_col = cpool.tile([128, TT], f32, tag="w_col")

            # --- projections per 512-token chunk ---
            for j in range(SC):
                slab = slab_pool.tile([128, DC * 512], bf16, tag="slab")
                nc.sync.dma_start(
                    out=slab[:],
                    in_=xt[:, j * 512:(j + 1) * 512].rearrange(
                        "(kd p) s -> p kd s", p=128),
                )
                slab_v = slab[:].rearrange("p (kd t) -> p kd t", kd=DC)

                psq = pq_pool.tile([QWK, 512], f32, tag="psq")
                for kd in range(DC):
                    nc.tensor.matmul(
                        psq[:],
                        wqwk_sb[:, kd * QWK:(kd + 1) * QWK],
                        slab_v[:, kd, :],
                        start=(kd == 0), stop=(kd == DC - 1),
                    )
                cols = slice(j * 512, (j + 1) * 512)
                # one VectorE copy drains q+w+k (peer chunks: q/w junk,
                # never read)
                nc.vector.tensor_copy(ab[0:48, cols], psq[0:48, :])
                if j < 4:
                    nc.vector.tensor_copy(wrow[0:1, cols], psq[64:65, :])
                # k band 0 replica via DMA (off the compute engines)
                nc.sync.dma_start(out=kt0[0:16, cols], in_=ab[32:48, cols])
                if j < 4:
                    # q band 1 replica
                    nc.sync.dma_start(
                        out=qb1[32:48, cols], in_=ab[0:16, cols],
                    )
                    # transpose gate pieces for this chunk's t-tiles
                    for t in range(4):
                        ti = j * 4 + t
                        nc.sync.dma_start(
                            out=w_col[:, ti:ti + 1],
                            in_=wrow[0:1, ti * 128:(ti + 1) * 128],
                        )

            # --- scores ---
            for ti in range(TT):
                osb = out_pool.tile([128, S], bf16, tag="osb")
                for pg in range(4):
                    pss = ps_pool.tile([128, 1024], f32, tag="pss")
                    for h in range(2):
                        sc = 2 * pg + h
                        if h == 0:
                            lhsT = ab[0:16, ti * 128:(ti + 1) * 128]
                            rhs = kt0[0:16, sc * 512:(sc + 1) * 512]
                        else:
                            lhsT = qb1[32:48, ti * 128:(ti + 1) * 128]
                            rhs = ab[32:48, sc * 512:(sc + 1) * 512]
                        nc.tensor.matmul(
                            pss[:, h * 512:(h + 1) * 512], lhsT, rhs,
                            start=True, stop=True,
                        )
                    oslice = osb[:, 2 * pg * 512:(2 * pg + 2) * 512]
                    if _S_PATH[ti * 4 + pg]:
                        nc.scalar.activation(
                            oslice, pss[:],
                            mybir.ActivationFunctionType.Relu,
                        )
                        nc.vector.tensor_scalar_mul(
                            out=oslice, in0=oslice,
                            scalar1=w_col[:, ti:ti + 1],
                        )
                    else:
                        nc.vector.tensor_scalar(
                            oslice, pss[:],
                            scalar1=0.0,
                            scalar2=w_col[:, ti:ti + 1],
                            op0=mybir.AluOpType.max,
                            op1=mybir.AluOpType.mult,
                        )
                nc.sync.dma_start(
                    out=o[ti * 128:(ti + 1) * 128, :],
                    in_=osb[:],
                )
    nc.compile()
    _CACHE["nc"] = nc
    return nc


def _make_in_maps(x, Wq, Wk, Ww):
    import ml_dtypes
    bf = ml_dtypes.bfloat16
    wqwk = np.zeros((D, QWK), dtype=np.float32)
    wqwk[:, 0:16] = Wq
    wqwk[:, 32:48] = Wk
    wqwk[:, 64:65] = Ww
    wqwk = np.ascontiguousarray(wqwk).astype(bf)
    xbf = x.astype(bf)
    in_maps = []
    for c in range(N_CORES):
        b, h = c // 2, c % 2
        own = xbf[b, h * T:(h + 1) * T, :]
        oth = xbf[b, (1 - h) * T:(2 - h) * T, :]
        xtc = np.ascontiguousarray(np.concatenate([own, oth], axis=0).T)
        in_maps.append({"xt": xtc, "wqwk": wqwk})
    return in_maps


def _assemble(results):
    out = np.empty((B, S, S), dtype=np.float32)
    for c in range(N_CORES):
        b, h = c // 2, c % 2
        oc = np.asarray(results[c]["o"], dtype=np.float32)
        if h == 1:
            oc = np.concatenate([oc[:, T:], oc[:, :T]], axis=1)
        out[b, h * T:(h + 1) * T, :] = oc
    return out


def kernel(x, Wq, Wk, Ww, _trace_kwargs=None):
    nc = _build_nc()
    in_maps = _make_in_maps(np.asarray(x, dtype=np.float32),
                            np.asarray(Wq, dtype=np.float32),
                            np.asarray(Wk, dtype=np.float32),
                            np.asarray(Ww, dtype=np.float32))
    kw = _trace_kwargs or {}
    res = run_bass_kernel_spmd(nc, in_maps, list(range(N_CORES)), **kw)
    out = _assemble(res.results)
    if _trace_kwargs is not None:
        return out, res
    return out
